# revision 24
# baseline (speedup 1.0000x reference)
"""DeltaRule (diagonal-state linear attention) Bass kernel for 8 TRN2 cores.

Problem: nn_DeltaRule_20194936225992
  B=4, S=2048, H_DIM=1024, N_HEADS=16, HEAD_DIM=64.
  q/k/v/b projections, phi = elu+1, per-(b,h,d) scalar linear recurrence
      s_t = (1 - b_t*pk_t^2) * s_{t-1} + b_t*v_t*pk_t ;  y_t = s_t * pq_t
  out = y @ Wo.T + bo

Sharding: core = (batch b, head-group hg) with hg covering 8 heads.
Each core computes its partial O-projection (contraction over its 512
lanes); host sums the two head-group partials per batch and adds bo.

On-device layout: lanes (h*64+d) on partitions, time on free dim.  The
recurrence runs as a hardware `tensor_tensor_scan` per [128,TC] tile,
chained across time chunks via the last column of the previous s.

Startup path (the PE floor is ~109us; everything else is edges):
  - dma_start triggers are ~600ns each on their issuing engine; the
    startup stream (~36 triggers) is split across Sync (k+v pieces),
    Scalar/ACT HWDGE (q pieces + early biases) and GpSimd SWDGE (bbb,
    bq, wo) so descriptor issue is not serialized on one queue.
  - chunk-0 projection loops run d-outer (4 psum banks live) so each
    arriving 2-dt piece feeds 8 matmuls immediately.
  - PE warmup matmuls (K=1, no memset dependency) start as soon as the
    Tensor queue is live; the PE clock ramps 1.2->2.4GHz ~7.1us after
    first activity (HAM), so early activity buys full clock sooner.
  - all HBM tensors are host-packed so every DMA descriptor moves >=2KB
    contiguous per partition.

O-projection is emitted transposed: psum tiles are [t,o] (lhsT = y
128-col slice, rhs = wo), so out rows are [S, H_DIM] with 2KB
contiguous rows, the final chunk's emission pipelines per 128-row
t-slice, and the host-side unshard needs no transpose.  Body copies
PSUM->SBUF ride ACT; the final chunk's copies split ACT/DVE and the
out dma triggers issue from the ACT queue right behind the copy.

Engine plan per chunk otherwise unchanged from the 132us baseline:
PE does Wq/Wk/Wv projections + previous chunk's O (software-pipelined
between the k and v phases); ACT does E=exp(x+b) for
phi(x)=min(exp(x),1+relu(x)) and the PSUM->SBUF O copies; DVE does the
rest of the elementwise chain and the hardware scan, all fp16 (2x DVE
mode).  The sigmoid gate b is computed on the host (0.4% of FLOPs) and
DMA'd pre-broadcast per lane.
"""

import os
import sys

for _p in ("/opt/trn_rl_repo", os.path.expanduser("~/.axon_site/_ro/trn_rl_repo")):
    if os.path.isdir(_p) and _p not in sys.path:
        sys.path.insert(0, _p)

import numpy as np  # noqa: E402

import concourse.bass as bass  # noqa: E402
import concourse.tile as tile  # noqa: E402
from concourse import bacc, mybir  # noqa: E402
from concourse.bass import ts  # noqa: E402
from concourse.bass_utils import run_bass_kernel_spmd  # noqa: E402

# problem constants (hardcoded per task rules)
B, S, H_DIM, N_HEADS, HEAD_DIM = 4, 2048, 1024, 16, 64
P = 128
NCORES = 8
HG = 2                      # head groups
J = 512                     # lanes per core  (8 heads * 64)
JT = J // P                 # 4 j-tiles
DT = H_DIM // P             # 8 contraction tiles
HPC = N_HEADS // HG         # 8 heads per core
TC = 512                    # time chunk (max PE moving dim / PSUM bank)
NCH = S // TC
# chunk-0 DMA piece plans (dt_start, n_dt), applied to both the weight
# and the x slice of each stream.  The DMA wire runs ~150/194/290/410
# GB/s for 1/2/4/8KB descriptors and chunk 0 is wire-rate-bound, BUT the
# PE consumes at only 1.2GHz until the HAM clock ramp (~7.1us after
# first continuous activity), so the k stream leads with small 2-dt
# (2KB) pieces to get the first matmul going early, then shifts to 4-dt
# (4KB) pieces for stream rate.
PLAN_K = [(0, 2), (2, 2), (4, 4)]
PLAN_VQ = [(0, 4), (4, 4)]


def _piece_map(plan):
    m = {}
    for i, (s, n) in enumerate(plan):
        for d in range(s, s + n):
            m[d] = (i, d - s)
    return m


D2P_K = _piece_map(PLAN_K)
D2P_VQ = _piece_map(PLAN_VQ)

F32 = mybir.dt.float32
F16 = mybir.dt.float16
AF = mybir.ActivationFunctionType
M = mybir.AluOpType

# env knobs for experiments
GP_WG = os.environ.get("DELTA_GP_WG", "0") != "0"   # w,g on GpSimd (slower)
PIPE_O = os.environ.get("DELTA_PIPE", "1") != "0"   # software-pipeline O-proj
N_WARM = int(os.environ.get("DELTA_WARM", "10"))    # PE p-state warmup matmuls
WMEMSET = os.environ.get("DELTA_WMEMSET", "1") != "0"  # memset warmup operands
PP_BUFS = int(os.environ.get("DELTA_PP", "5"))      # pproj PSUM banks
PO_BUFS = int(os.environ.get("DELTA_PO", "3"))      # O-proj PSUM banks


def build_nc(with_vbias):
    nc = bacc.Bacc(trn_type="TRN2", target_bir_lowering=False, debug=False)

    # per-core inputs, host-packed (see make_in_maps)
    xq = nc.dram_tensor("xq", [P, NCH, DT, TC], F16, kind="ExternalInput").ap()
    xk = nc.dram_tensor("xk", [P, NCH, DT, TC], F16, kind="ExternalInput").ap()
    xv = nc.dram_tensor("xv", [P, NCH, DT, TC], F16, kind="ExternalInput").ap()
    bbb = nc.dram_tensor("bbb", [P, NCH, JT, TC], F16, kind="ExternalInput").ap()
    wq = nc.dram_tensor("wq", [P, DT, J], F16, kind="ExternalInput").ap()
    wk = nc.dram_tensor("wk", [P, DT, J], F16, kind="ExternalInput").ap()
    wv = nc.dram_tensor("wv", [P, DT, J], F16, kind="ExternalInput").ap()
    wo = nc.dram_tensor("wo", [P, JT, H_DIM], F16, kind="ExternalInput").ap()
    bq = nc.dram_tensor("bq", [P, JT], F32, kind="ExternalInput").ap()
    bk = nc.dram_tensor("bk", [P, JT], F32, kind="ExternalInput").ap()
    bq1 = nc.dram_tensor("bq1", [P, JT], F32, kind="ExternalInput").ap()
    bk1 = nc.dram_tensor("bk1", [P, JT], F32, kind="ExternalInput").ap()
    if with_vbias:
        bvr = nc.dram_tensor("bvr", [1, J], F16, kind="ExternalInput").ap()
    out = nc.dram_tensor("out", [S, H_DIM], F16, kind="ExternalOutput").ap()

    from contextlib import ExitStack

    with tile.TileContext(nc) as tcx, ExitStack() as ctx:
        wpool = ctx.enter_context(tcx.tile_pool(name="weights", bufs=1))
        c0pool = ctx.enter_context(tcx.tile_pool(name="c0x", bufs=1))
        xpool = ctx.enter_context(tcx.tile_pool(name="xin", bufs=2))
        ipool = ctx.enter_context(tcx.tile_pool(name="inter", bufs=3))
        spool = ctx.enter_context(tcx.tile_pool(name="scan", bufs=2))
        opool = ctx.enter_context(tcx.tile_pool(name="osb", bufs=4))
        pproj = ctx.enter_context(
            tcx.tile_pool(name="pproj", bufs=PP_BUFS, space="PSUM"))
        po = ctx.enter_context(tcx.tile_pool(name="po", bufs=PO_BUFS, space="PSUM"))

        # --- PE p-state warmup: K=1 matmuls keep the PE busy through the
        # DMA-bound preamble; HAM (full 2.4GHz clock) arrives ~7.1us after
        # the first PE activity, so these start with no data dependency
        # (operands left uninitialized unless DELTA_WMEMSET) ---
        if N_WARM:
            wa = wpool.tile([1, P], F16, tag="warm_a")
            wb = wpool.tile([1, TC], F16, tag="warm_b")
            if WMEMSET:
                nc.vector.memset(wa[:], 0.0)
                nc.vector.memset(wb[:], 0.0)
            pwarm = po.tile([P, TC], F32, tag="po", name="pwarm")
            for _ in range(N_WARM):
                nc.tensor.matmul(out=pwarm[:], lhsT=wa[:], rhs=wb[:],
                                 start=True, stop=True)

        # --- persistent weights / constants (per-piece tiles) ---
        def wpieces(tag, plan):
            return [wpool.tile([P, n, J], F16, tag=f"{tag}{i}", name=f"{tag}{i}")
                    for i, (_, n) in enumerate(plan)]

        wk_p = wpieces("wk", PLAN_K)
        wv_p = wpieces("wv", PLAN_VQ)
        wq_p = wpieces("wq", PLAN_VQ)
        wo_sb = wpool.tile([P, JT, H_DIM], F16, tag="wo")
        bq_sb = wpool.tile([P, JT], F32, tag="bq")
        bk_sb = wpool.tile([P, JT], F32, tag="bk")
        bq1_sb = wpool.tile([P, JT], F32, tag="bq1")
        bk1_sb = wpool.tile([P, JT], F32, tag="bk1")
        if with_vbias:
            bvr_sb = wpool.tile([1, J], F16, tag="bvr")
            ones_sb = wpool.tile([1, TC], F16, tag="ones")

        # chunk-0 x pieces (separate tiles so matmuls start per-piece)
        def xpieces(tag, plan):
            return [c0pool.tile([P, n, TC], F16, tag=f"{tag}{i}", name=f"{tag}{i}")
                    for i, (_, n) in enumerate(plan)]

        xk0_p = xpieces("xk0", PLAN_K)
        xv0_p = xpieces("xv0", PLAN_VQ)
        xq0_p = xpieces("xq0", PLAN_VQ)

        def wsrc(w_p, d2p, d, jsl):
            i, off = d2p[d]
            return w_p[i][:, off, jsl]

        # --- startup DMA: ALL bulk transfers on the Sync queue in strict
        # consumption order (the 16 DMA engines round-robin across queues,
        # so a second bulk queue steals bandwidth from the piece the PE
        # needs first — measured as a 3x slowdown of the k stream).  bbb
        # goes last: its only consumer (w = pk*b on DVE) can stall without
        # holding up any matmul.  Only the tiny bias vectors ride the
        # Scalar HWDGE queue. ---
        def wdma(w_p, wt, plan, i):
            s, n = plan[i]
            nc.sync.dma_start(out=w_p[i][:], in_=wt[:, s:s + n, :])

        def xdma(x_p, xt, plan, i):
            s, n = plan[i]
            nc.sync.dma_start(out=x_p[i][:], in_=xt[:, 0, s:s + n, :])

        for i in range(len(PLAN_K)):
            wdma(wk_p, wk, PLAN_K, i)
            xdma(xk0_p, xk, PLAN_K, i)
        for i in range(len(PLAN_VQ)):
            wdma(wv_p, wv, PLAN_VQ, i)
            xdma(xv0_p, xv, PLAN_VQ, i)
        for i in range(len(PLAN_VQ)):
            wdma(wq_p, wq, PLAN_VQ, i)
            xdma(xq0_p, xq, PLAN_VQ, i)
        bb_c = xpool.tile([P, JT, TC], F16, tag="bbb")
        nc.sync.dma_start(out=bb_c[:], in_=bbb[:, 0])
        # Scalar (ACT HWDGE): the bias columns (a few KB, instant)
        nc.scalar.dma_start(out=bk_sb[:], in_=bk)
        nc.scalar.dma_start(out=bk1_sb[:], in_=bk1)
        nc.scalar.dma_start(out=bq_sb[:], in_=bq)
        nc.scalar.dma_start(out=bq1_sb[:], in_=bq1)
        if with_vbias:
            nc.scalar.dma_start(out=bvr_sb[:], in_=bvr)
            nc.vector.memset(ones_sb[:], 1.0)
        # wo is dispatched on Sync after chunk-1's x prefetch (first needed
        # by the pipelined O(0) midway through chunk 1)

        s_prev = [None] * JT    # last-chunk scan state tile per lane-tile
        y_prev = [None] * JT    # previous chunk's y tiles (for pipelined O)

        eng_wg = nc.gpsimd if GP_WG else nc.vector

        def emit_O(cp, y_tiles, final=False):
            """Transposed O-projection of chunk cp:
            out[t, o] += y[j, t] * wo[j, o], per 128-row t-slice.

            Copies ride ACT (idle slack in the body); the final chunk's
            second half goes to DVE so the two copies of each t-slice run
            in parallel and the tail is one copy + one dma deep."""
            for tsl in range(JT):
                o_sb = opool.tile([P, H_DIM], F16, tag="osb")
                for oh in range(2):
                    pso = po.tile([P, TC], F32, tag="po")
                    for lt in range(JT):
                        nc.tensor.matmul(
                            out=pso[:], lhsT=y_tiles[lt][:, ts(tsl, P)],
                            rhs=wo_sb[:, lt, ts(oh, TC)],
                            start=(lt == 0), stop=(lt == JT - 1),
                        )
                    if final and oh == 1:
                        nc.vector.tensor_copy(out=o_sb[:, ts(oh, TC)], in_=pso[:])
                    else:
                        nc.scalar.copy(out=o_sb[:, ts(oh, TC)], in_=pso[:])
                r0 = cp * TC + tsl * P
                nc.scalar.dma_start(out=out[r0:r0 + P, :], in_=o_sb[:])

        for c in range(NCH):
            # --- stream x chunk (c>0); chunk 0 was sliced above ---
            if c > 0:
                xk_c = xpool.tile([P, DT, TC], F16, tag="xk")
                nc.sync.dma_start(out=xk_c[:], in_=xk[:, c])
                xv_c = xpool.tile([P, DT, TC], F16, tag="xv")
                nc.sync.dma_start(out=xv_c[:], in_=xv[:, c])
                bb_c = xpool.tile([P, JT, TC], F16, tag="bbb")
                nc.sync.dma_start(out=bb_c[:], in_=bbb[:, c])
                xq_c = xpool.tile([P, DT, TC], F16, tag="xq")
                nc.sync.dma_start(out=xq_c[:], in_=xq[:, c])
                if c == 1:
                    nc.sync.dma_start(out=wo_sb[:], in_=wo)

            def xsrc(whole, pieces, d2p, d):
                if c == 0:
                    i, off = d2p[d]
                    return pieces[i][:, off, :]
                return whole[:, d, :]

            # ---- k projections + phi(k) + scan coefficients ----
            # chunk 0 runs d-outer so each arriving piece feeds 4 matmuls
            pk_t, w_t = [], []
            if c == 0:
                psk_t = [pproj.tile([P, TC], F32, tag="proj", name=f"psk{lt}")
                         for lt in range(JT)]
                for d in range(DT):
                    for lt in range(JT):
                        nc.tensor.matmul(
                            out=psk_t[lt][:], lhsT=wsrc(wk_p, D2P_K, d, ts(lt, P)),
                            rhs=xsrc(None, xk0_p, D2P_K, d),
                            start=(d == 0), stop=(d == DT - 1),
                        )
            for lt in range(JT):
                jsl = ts(lt, P)
                if c == 0:
                    psk = psk_t[lt]
                else:
                    psk = pproj.tile([P, TC], F32, tag="proj")
                    for d in range(DT):
                        nc.tensor.matmul(
                            out=psk[:], lhsT=wsrc(wk_p, D2P_K, d, jsl),
                            rhs=xsrc(xk_c, None, None, d),
                            start=(d == 0), stop=(d == DT - 1),
                        )
                # phi(x) = min(exp(x), 1 + relu(x)), exact
                ek = ipool.tile([P, TC], F16, tag="ek")
                nc.scalar.activation(out=ek[:], in_=psk[:], func=AF.Exp,
                                     bias=bk_sb[:, lt:lt + 1])
                uk = ipool.tile([P, TC], F16, tag="uk")
                nc.vector.tensor_scalar(
                    out=uk[:], in0=psk[:], scalar1=bk1_sb[:, lt:lt + 1],
                    scalar2=1.0, op0=M.add, op1=M.max)
                pk = ipool.tile([P, TC], F16, tag="pk")
                nc.vector.tensor_tensor(out=pk[:], in0=uk[:], in1=ek[:], op=M.min)
                pk_t.append(pk)
                w = ipool.tile([P, TC], F16, tag="w")
                eng_wg.tensor_tensor(out=w[:], in0=pk[:], in1=bb_c[:, lt, :],
                                     op=M.mult)
                w_t.append(w)

            # ---- previous chunk's O-projection (fills the PE while this
            # chunk's elementwise chain completes) ----
            if PIPE_O and c > 0:
                emit_O(c - 1, y_prev)

            # ---- v projections + scan ----
            s_new_t = []
            psv_t = [None] * JT
            if c == 0:
                # lt2/lt3 psum from the (still unused) po pool so the PE
                # doesn't stall waiting for k banks to be read out; lt1 is
                # ordered last within each d to give ek0/uk0 time to free
                # psk0 for it
                vorder = (0, 2, 3, 1)
                for lt in (0, 1):
                    psv_t[lt] = pproj.tile([P, TC], F32, tag="proj",
                                           name=f"psv{lt}")
                for lt in (2, 3):
                    psv_t[lt] = po.tile([P, TC], F32, tag="po",
                                        name=f"psv{lt}")
                for d in range(DT):
                    for lt in vorder:
                        nc.tensor.matmul(
                            out=psv_t[lt][:], lhsT=wsrc(wv_p, D2P_VQ, d, ts(lt, P)),
                            rhs=xsrc(None, xv0_p, D2P_VQ, d),
                            start=(d == 0),
                            stop=(d == DT - 1) and not with_vbias,
                        )
            for lt in range(JT):
                jsl = ts(lt, P)
                if c == 0:
                    psv = psv_t[lt]
                else:
                    psv = pproj.tile([P, TC], F32, tag="proj")
                    for d in range(DT):
                        nc.tensor.matmul(
                            out=psv[:], lhsT=wsrc(wv_p, D2P_VQ, d, jsl),
                            rhs=xsrc(xv_c, None, None, d),
                            start=(d == 0),
                            stop=(d == DT - 1) and not with_vbias,
                        )
                if with_vbias:
                    nc.tensor.matmul(out=psv[:], lhsT=bvr_sb[:, jsl],
                                     rhs=ones_sb[:], start=False, stop=True)
                pk, w = pk_t[lt], w_t[lt]
                g = ipool.tile([P, TC], F16, tag="g")
                eng_wg.tensor_tensor(out=g[:], in0=pk[:], in1=w[:], op=M.mult)
                a = ipool.tile([P, TC], F16, tag="a")
                nc.vector.tensor_scalar(out=a[:], in0=g[:], scalar1=-1.0,
                                        scalar2=1.0, op0=M.mult, op1=M.add)
                cc = ipool.tile([P, TC], F16, tag="cc")
                nc.vector.tensor_tensor(out=cc[:], in0=psv[:], in1=w[:], op=M.mult)
                s_new = spool.tile([P, TC], F16, tag=f"s{lt}")
                init = 0.0 if c == 0 else s_prev[lt][:, TC - 1:TC]
                nc.vector.tensor_tensor_scan(
                    out=s_new[:], data0=a[:], data1=cc[:], initial=init,
                    op0=M.mult, op1=M.add,
                )
                s_prev[lt] = s_new
                s_new_t.append(s_new)

            # ---- q projections + phi(q) + y = s * pq ----
            y_new = []
            psq_t = [None] * JT
            if c == 0:
                for lt in range(JT):
                    psq_t[lt] = pproj.tile([P, TC], F32, tag="proj",
                                           name=f"psq{lt}")
                for d in range(DT):
                    for lt in range(JT):
                        nc.tensor.matmul(
                            out=psq_t[lt][:], lhsT=wsrc(wq_p, D2P_VQ, d, ts(lt, P)),
                            rhs=xsrc(None, xq0_p, D2P_VQ, d),
                            start=(d == 0), stop=(d == DT - 1),
                        )
            for lt in range(JT):
                jsl = ts(lt, P)
                if c == 0:
                    psq = psq_t[lt]
                else:
                    psq = pproj.tile([P, TC], F32, tag="proj")
                    for d in range(DT):
                        nc.tensor.matmul(
                            out=psq[:], lhsT=wsrc(wq_p, D2P_VQ, d, jsl),
                            rhs=xsrc(xq_c, None, None, d),
                            start=(d == 0), stop=(d == DT - 1),
                        )
                eq = ipool.tile([P, TC], F16, tag="ek")
                nc.scalar.activation(out=eq[:], in_=psq[:], func=AF.Exp,
                                     bias=bq_sb[:, lt:lt + 1])
                uq = ipool.tile([P, TC], F16, tag="uk")
                nc.vector.tensor_scalar(
                    out=uq[:], in0=psq[:], scalar1=bq1_sb[:, lt:lt + 1],
                    scalar2=1.0, op0=M.add, op1=M.max)
                pq = ipool.tile([P, TC], F16, tag="pk")
                nc.vector.tensor_tensor(out=pq[:], in0=uq[:], in1=eq[:], op=M.min)
                y = spool.tile([P, TC], F16, tag=f"y{lt}")
                nc.vector.tensor_tensor(out=y[:], in0=s_new_t[lt][:], in1=pq[:],
                                        op=M.mult)
                y_new.append(y)
            y_prev = y_new

            if not PIPE_O:
                emit_O(c, y_prev)

        if PIPE_O:
            emit_O(NCH - 1, y_prev, final=True)

    nc.compile()
    return nc


_NC_CACHE = {}


def _get_nc(with_vbias):
    key = (with_vbias, GP_WG, PIPE_O, N_WARM, WMEMSET, PP_BUFS, PO_BUFS)
    if key not in _NC_CACHE:
        _NC_CACHE[key] = build_nc(with_vbias)
    return _NC_CACHE[key]


def make_in_maps(query, key, value, beta, Wq, bq, Wk, bk, Wv, bv, Wb, bb, Wo, bo,
                 with_vbias):
    """Host-side shard prep: core_id = b*2 + hg."""

    def xpack(x):  # [S, H_DIM] -> [p, ch, dt, t] fp16 (8KB/partition/chunk)
        a = np.asarray(x, np.float32).T.reshape(DT, P, NCH, TC)
        return np.ascontiguousarray(a.transpose(1, 2, 0, 3)).astype(np.float16)

    def wpackT(Wsl):  # [J, H_DIM] -> [p, dt, j] fp16 (contiguous dt slices)
        a = np.asarray(Wsl, np.float32).T.reshape(DT, P, J)
        return np.ascontiguousarray(a.transpose(1, 0, 2)).astype(np.float16)

    def wopack(Wosl):  # [H_DIM, J] -> [p, jt, o] fp16 (8KB/partition)
        a = np.asarray(Wosl, np.float32).T.reshape(JT, P, H_DIM)
        return np.ascontiguousarray(a.transpose(1, 0, 2)).astype(np.float16)

    xqs = [xpack(query[b]) for b in range(B)]
    xks = [xpack(key[b]) for b in range(B)]
    xvs = [xpack(value[b]) for b in range(B)]
    # gate b computed host-side (0.4% of FLOPs), pre-broadcast per lane
    Wbf = np.asarray(Wb, np.float32)
    bbf0 = np.asarray(bb, np.float32)
    z = np.einsum('bsd,hd->bsh', np.asarray(beta, np.float32), Wbf) + bbf0
    bgate = 1.0 / (1.0 + np.exp(-z))                      # [B, S, 16]

    bqf = np.asarray(bq, np.float32)
    bkf = np.asarray(bk, np.float32)
    bvf = np.asarray(bv, np.float32)

    in_maps = []
    for b in range(B):
        for hg in range(HG):
            jsl = slice(hg * J, (hg + 1) * J)
            hsl = slice(hg * HPC, (hg + 1) * HPC)

            def lanes(v):  # [J] -> [128, 4] per lane-tile columns
                return np.ascontiguousarray(v[jsl].reshape(JT, P).T)

            # [S, 512] lane-broadcast gate -> [p, ch, lt, t]
            rep = np.repeat(bgate[b][:, hsl], HEAD_DIM, axis=1).T  # [512, S]
            bl = np.ascontiguousarray(
                rep.reshape(JT, P, NCH, TC).transpose(1, 2, 0, 3)
            ).astype(np.float16)

            m = {
                "xq": xqs[b], "xk": xks[b], "xv": xvs[b], "bbb": bl,
                "wq": wpackT(Wq[jsl]), "wk": wpackT(Wk[jsl]),
                "wv": wpackT(Wv[jsl]), "wo": wopack(Wo[:, jsl]),
                "bq": lanes(bqf), "bk": lanes(bkf),
                "bq1": lanes(bqf) + 1.0, "bk1": lanes(bkf) + 1.0,
            }
            if with_vbias:
                m["bvr"] = bvf[jsl].reshape(1, J).astype(np.float16)
            in_maps.append(m)
    return in_maps


LAST_RESULTS = None


def kernel(**inputs):
    global LAST_RESULTS
    with_vbias = bool(np.any(np.asarray(inputs["bv"], np.float32)))
    nc = _get_nc(with_vbias)
    in_maps = make_in_maps(**inputs, with_vbias=with_vbias)
    res = run_bass_kernel_spmd(nc, in_maps, core_ids=list(range(NCORES)),
                               trace=bool(os.environ.get("DELTA_TRACE")))
    LAST_RESULTS = res
    bo = np.asarray(inputs["bo"], np.float32)
    out = np.empty((B, S, H_DIM), np.float32)
    for b in range(B):
        out[b] = (res.results[2 * b]["out"].astype(np.float32)
                  + res.results[2 * b + 1]["out"].astype(np.float32)) + bo
    return out
